# revision 14
# baseline (speedup 1.0000x reference)
"""Trainium2 Bass kernel for the AttentionBlock problem.

Reference semantics (shapes hardcoded):
    x [4, 256, 64, 64]; 1x1-conv weights q_w/k_w/v_w [256, 258] (+biases),
    fc_w [256, 256], fc_b [256].
    x0 = concat(x, pos) -> [B, 258, 4096]
    q/k/v = relu(W @ x0 + b)                    [B, 256, 4096]
    attn  = softmax_causal(q^T k)               [B, 4096, 4096]
    out   = x + relu(fc_w @ (attn @ v^T)^T + fc_b)

Distribution: 8 cores = 4 batches x 2 query-block roles. Each core
computes full k / v^T for its batch, q only for its 4 owned 512-wide
query blocks, and causal attention for those blocks. Causal work is
balanced by giving role 0 global blocks [0,3,4,7] and role 1 blocks
[1,2,5,6]; both roles run the identical SPMD program with per-slot
key-tile counts [8,16,24,32] (slightly padded), with per-core mask
data zeroing padded/non-causal entries.

Per-slot band masks take only two canonical [8,128,512] patterns:
H = [1,1,1,1,T0..T3] (slot is exact) and L = [T0..T3,0,0,0,0] (slot
is padded by 4 tiles); for both roles the pattern alternates with
slot parity, so per-core mask data is just [2, 8, 128, 512] indexed
by s%2 (H/L order role-dependent), resident in SBUF.

Softmax is computed without max-subtraction (scores are ~26+-5, far
from fp32 overflow): p = exp(s) * mask, normalized by a replicated
ones-matmul denominator (the [128,128] ones lhsT yields the column
sums broadcast across all partitions). Per-key-tile exps are summed
on VectorE (bf16 quad sums + f32 running total), so only ONE ones
matmul per slot runs on the PE.

Precision split: the score path (q/k projections, q^T k) runs in
float32r; everything whose error enters linearly (exp(p) weights, v,
fc, masks) runs in bf16. Channel-half pairs of x/weights are packed
host-side into [128, 2, *] arrays so each k/v/q input tile is one
DMA. The pos rows (px/py/bias) live in resident [3, N] tiles; the
pos+bias contribution of each projection is a 3-row matmul.

Emission interleaves phases (pair0, q0, slot0, pair1, q1, slot1, ...)
so the PE always has runway while later pair DMAs land. v relus run
on VectorE to keep ScalarE off the phase-A critical path.
"""

import numpy as np

B = 4
C = 256
S = 64
N = S * S            # 4096
K = 256              # q/k/v channels
NBLK = 512           # query block width
NSLOT = 4            # owned query blocks per core
M_S = (8, 16, 24, 32)  # key-tile count per slot (128-wide key tiles)
BLOCKS = ((0, 3, 4, 7), (1, 2, 5, 6))  # role -> global block ids

_PROGRAM = None


def _build_program():
    import concourse.bacc as bacc
    import concourse.mybir as mybir
    import concourse.tile as tile

    F32 = mybir.dt.float32
    F32R = mybir.dt.float32r
    BF16 = mybir.dt.bfloat16
    Act = mybir.ActivationFunctionType

    nc = bacc.Bacc("TRN2", target_bir_lowering=False, debug=False)

    # channel-half-merged inputs: [...][h][...] picks channel half h
    xf_d = nc.dram_tensor("xf", [128, 2, N], F32R, kind="ExternalInput")
    xb_d = nc.dram_tensor("xb", [128, 2, N], BF16, kind="ExternalInput")
    xq_d = nc.dram_tensor("xq", [128, 2, NSLOT * NBLK], F32R,
                          kind="ExternalInput")
    p3f_d = nc.dram_tensor("p3f", [3, N], F32R, kind="ExternalInput")
    p3b_d = nc.dram_tensor("p3b", [3, N], BF16, kind="ExternalInput")
    p3q_d = nc.dram_tensor("p3q", [3, NSLOT * NBLK], F32R,
                           kind="ExternalInput")
    wq_d = nc.dram_tensor("wq", [128, 2, K], F32R, kind="ExternalInput")
    wk_d = nc.dram_tensor("wk", [128, 2, K], F32R, kind="ExternalInput")
    wv_d = nc.dram_tensor("wv", [128, 2, K], BF16, kind="ExternalInput")
    wq3_d = nc.dram_tensor("wq3", [3, K], F32R, kind="ExternalInput")
    wk3_d = nc.dram_tensor("wk3", [3, K], F32R, kind="ExternalInput")
    wv3_d = nc.dram_tensor("wv3", [3, K], BF16, kind="ExternalInput")
    fcw_d = nc.dram_tensor("fcw", [128, 2, C], BF16, kind="ExternalInput")
    fcb_d = nc.dram_tensor("fcb", [128, 2], F32, kind="ExternalInput")
    msk_d = nc.dram_tensor("masks", [128, 2, 8, NBLK], BF16,
                           kind="ExternalInput")
    ob_d = nc.dram_tensor("ones_b", [128, 128], F32R, kind="ExternalInput")
    out_d = nc.dram_tensor("out", [C, NSLOT * NBLK], F32, kind="ExternalOutput")

    with tile.TileContext(nc) as tc:
        with (
            tc.tile_pool(name="wts", bufs=1) as wts,
            tc.tile_pool(name="pos_p", bufs=3) as pos_p,
            tc.tile_pool(name="x0_p", bufs=4) as x0_p,
            tc.tile_pool(name="xq_p", bufs=1) as xq_p,
            tc.tile_pool(name="kqv_p", bufs=1) as kqv_p,
            tc.tile_pool(name="msk_p", bufs=1) as msk_p,
            tc.tile_pool(name="ex_p", bufs=9) as ex_p,
            tc.tile_pool(name="ds_p", bufs=3) as ds_p,
            tc.tile_pool(name="tot_p", bufs=2) as tot_p,
            tc.tile_pool(name="o_p", bufs=4) as o_p,
            tc.tile_pool(name="rb_p", bufs=2) as rb_p,
            tc.tile_pool(name="tr_p", bufs=2) as tr_p,
            tc.tile_pool(name="ps_sc", bufs=5, space="PSUM") as ps_sc,
            tc.tile_pool(name="ps_out", bufs=1, space="PSUM") as ps_out,
            tc.tile_pool(name="ps_mx", bufs=1, space="PSUM") as ps_mx,
        ):
            def wtile(dram, shape, dt, tag):
                t = wts.tile(shape, dt, tag=tag, name=tag)
                nc.sync.dma_start(t[:], dram[:])
                return t

            k_sb = [[None] * 8 for _ in range(2)]
            vT_sb = [None] * 32
            q_sb = [[None] * NSLOT for _ in range(2)]

            def emit_pair_dmas(nbp):
                # k inputs (xf) for both blocks first, then v inputs (xb)
                xfs, xbs = [], []
                for nb in (2 * nbp, 2 * nbp + 1):
                    sl = slice(NBLK * nb, NBLK * (nb + 1))
                    xf = x0_p.tile([128, 2, NBLK], F32R, tag="xf",
                                   name=f"xf_{nb}")
                    nc.sync.dma_start(xf[:], xf_d[:, :, sl])
                    pf = pos_p.tile([3, NBLK], F32R, tag="p3f",
                                    name=f"p3f_{nb}")
                    nc.sync.dma_start(pf[:], p3f_d[:, sl])
                    xfs.append((xf, pf))
                for nb in (2 * nbp, 2 * nbp + 1):
                    sl = slice(NBLK * nb, NBLK * (nb + 1))
                    xb = x0_p.tile([128, 2, NBLK], BF16, tag="xb",
                                   name=f"xb_{nb}")
                    nc.sync.dma_start(xb[:], xb_d[:, :, sl])
                    pb = pos_p.tile([3, NBLK], BF16, tag="p3b",
                                    name=f"p3b_{nb}")
                    nc.sync.dma_start(pb[:], p3b_d[:, sl])
                    xbs.append((xb, pb))
                return list(zip(xfs, xbs))

            def emit_pair_compute(nbp, tiles):
                for li, nb in enumerate((2 * nbp, 2 * nbp + 1)):
                    (xf, pf), _ = tiles[li]
                    for kt in range(2):
                        kts = slice(128 * kt, 128 * (kt + 1))
                        pk = ps_sc.tile([128, NBLK], F32, tag="sc",
                                        name=f"pk{kt}_{nb}")
                        nc.tensor.matmul(pk[:], wk_t[:, 0, kts], xf[:, 0, :],
                                         start=True, stop=False)
                        nc.tensor.matmul(pk[:], wk_t[:, 1, kts], xf[:, 1, :],
                                         start=False, stop=False)
                        nc.tensor.matmul(pk[:], wk3_t[:, kts], pf[:],
                                         start=False, stop=True)
                        kt_sb = kqv_p.tile([128, NBLK], F32R,
                                           tag=f"k{kt}_{nb}",
                                           name=f"k{kt}_{nb}")
                        nc.scalar.activation(kt_sb[:], pk[:], Act.Relu)
                        k_sb[kt][nb] = kt_sb
                for li, nb in enumerate((2 * nbp, 2 * nbp + 1)):
                    _, (xb, pb) = tiles[li]
                    for sub in range(4):
                        i = 4 * nb + sub
                        ss = slice(128 * sub, 128 * (sub + 1))
                        pv = ps_sc.tile([128, NBLK], F32, tag="sc",
                                        name=f"pv{i}")
                        nc.tensor.matmul(pv[:, :K], xb[:, 0, ss], wv_t[:, 0, :],
                                         start=True, stop=False)
                        nc.tensor.matmul(pv[:, :K], xb[:, 1, ss], wv_t[:, 1, :],
                                         start=False, stop=False)
                        nc.tensor.matmul(pv[:, :K], pb[:, ss], wv3_t[:],
                                         start=False, stop=True)
                        vt_sb = kqv_p.tile([128, K], BF16, tag=f"v{i}",
                                           name=f"v{i}")
                        nc.vector.tensor_scalar_max(vt_sb[:], pv[:, :K], 0.0)
                        vT_sb[i] = vt_sb

            def emit_q(s, xqs):
                sl = slice(NBLK * s, NBLK * (s + 1))
                for kt in range(2):
                    kts = slice(128 * kt, 128 * (kt + 1))
                    pq = ps_sc.tile([128, NBLK], F32, tag="sc",
                                    name=f"pq{kt}_{s}")
                    nc.tensor.matmul(pq[:], wq_t[:, 0, kts], xqs[:, 0, :],
                                     start=True, stop=False)
                    nc.tensor.matmul(pq[:], wq_t[:, 1, kts], xqs[:, 1, :],
                                     start=False, stop=False)
                    nc.tensor.matmul(pq[:], wq3_t[:, kts], p3q_t[:, sl],
                                     start=False, stop=True)
                    qt = kqv_p.tile([128, NBLK], F32R, tag=f"q{kt}_{s}",
                                    name=f"q{kt}_{s}")
                    nc.scalar.activation(qt[:], pq[:], Act.Relu)
                    q_sb[kt][s] = qt

            def finalize_slot(s, po, pd, xqs):
                """normalize slot s, fc, relu, residual, dma out."""
                rb_sb = rb_p.tile([128, NBLK], F32, tag="rb", name=f"rb{s}")
                nc.vector.reciprocal_approx_fast(rb_sb[:], pd[:])
                o_sb = []
                for vt in range(2):
                    ot = o_p.tile([128, NBLK], BF16, tag="o",
                                  name=f"o{vt}_{s}")
                    nc.vector.tensor_mul(ot[:], po[vt][:], rb_sb[:])
                    o_sb.append(ot)
                for ot in range(2):
                    pfc = ps_mx.tile([128, NBLK], F32, tag="mx",
                                     name=f"pfc{ot}_{s}")
                    for vt in range(2):
                        nc.tensor.matmul(
                            pfc[:], fcw_t[:, vt, 128 * ot:128 * (ot + 1)],
                            o_sb[vt][:], start=(vt == 0), stop=(vt == 1))
                    t_sb = tr_p.tile([128, NBLK], F32, tag=f"t{ot}",
                                     name=f"t{ot}_{s}")
                    nc.scalar.activation(t_sb[:], pfc[:], Act.Relu,
                                         bias=fcb_t[:, ot:ot + 1])
                    r_sb = tr_p.tile([128, NBLK], F32, tag=f"r{ot}",
                                     name=f"r{ot}_{s}")
                    nc.vector.tensor_add(r_sb[:], t_sb[:], xqs[:, ot, :])
                    nc.sync.dma_start(
                        out_d[128 * ot:128 * (ot + 1),
                              NBLK * s:NBLK * (s + 1)], r_sb[:])

            def emit_slot(s, fin):
                M = M_S[s]
                po = [ps_out.tile([128, NBLK], F32, tag=f"o{vt}",
                                  name=f"po{vt}_{s}") for vt in range(2)]
                ex_tiles = [None] * M
                tot = [None]

                def emit_scores(i):
                    # scores^T tile [128 keys, 512 queries]
                    psc = ps_sc.tile([128, NBLK], F32, tag="sc",
                                     name=f"psc{s}_{i}")
                    for kt in range(2):
                        nc.tensor.matmul(
                            psc[:],
                            k_sb[kt][i // 4][:, 128 * (i % 4):128 * (i % 4 + 1)],
                            q_sb[kt][s][:], start=(kt == 0), stop=(kt == 1))
                    ex = ex_p.tile([128, NBLK], BF16, tag="ex",
                                   name=f"ex{s}_{i}")
                    nc.scalar.activation(ex[:], psc[:], Act.Exp)
                    if i >= M - 8:
                        nc.vector.tensor_mul(
                            ex[:], ex[:], msk_t[:, s % 2, i - (M - 8), :])
                    ex_tiles[i] = ex

                def consume_quad(j):
                    for jj in range(j, j + 4):
                        e = ex_tiles[jj]
                        for vt in range(2):
                            nc.tensor.matmul(
                                po[vt][:],
                                vT_sb[jj][:, 128 * vt:128 * (vt + 1)],
                                e[:], start=(jj == 0), stop=(jj == M - 1))
                    # bf16 quad sum, then f32 running total on VectorE
                    da = ds_p.tile([128, NBLK], BF16, tag="ds",
                                   name=f"da{s}_{j}")
                    nc.vector.tensor_add(da[:], ex_tiles[j][:],
                                         ex_tiles[j + 1][:])
                    db = ds_p.tile([128, NBLK], BF16, tag="ds",
                                   name=f"db{s}_{j}")
                    nc.vector.tensor_add(db[:], ex_tiles[j + 2][:],
                                         ex_tiles[j + 3][:])
                    t = tot_p.tile([128, NBLK], F32R, tag="tot",
                                   name=f"tot{s}_{j}")
                    if tot[0] is None:
                        nc.vector.tensor_add(t[:], da[:], db[:])
                    else:
                        dsum = ds_p.tile([128, NBLK], BF16, tag="ds",
                                         name=f"ds{s}_{j}")
                        nc.vector.tensor_add(dsum[:], da[:], db[:])
                        nc.vector.tensor_add(t[:], tot[0][:], dsum[:])
                    tot[0] = t
                    for jj in range(j, j + 4):
                        ex_tiles[jj] = None

                # 4-tile score batches between bf16 consume batches; the
                # previous slot's finalize is deferred to after the second
                # score batch so VectorE has runway for recip+muls
                for ib in range(0, M, 4):
                    for i in range(ib, ib + 4):
                        emit_scores(i)
                    if ib == 4 and fin is not None:
                        finalize_slot(*fin)
                    if ib >= 4:
                        consume_quad(ib - 4)
                consume_quad(M - 4)
                # single denominator matmul per slot
                pd = ps_mx.tile([128, NBLK], F32, tag="mx", name=f"pd{s}")
                nc.tensor.matmul(pd[:], ones_b[:], tot[0][:],
                                 start=True, stop=True)
                return po, pd

            # ---------------- emission schedule ----------------
            # head DMAs in first-consumer order: k weights, pair-0 xf,
            # v weights, pair-0 xb
            wk_t = wtile(wk_d, [128, 2, K], F32R, "wk")
            wk3_t = wtile(wk3_d, [3, K], F32R, "wk3")
            xf0s = []
            for nb in (0, 1):
                sl = slice(NBLK * nb, NBLK * (nb + 1))
                xf = x0_p.tile([128, 2, NBLK], F32R, tag="xf", name=f"xf_{nb}")
                nc.sync.dma_start(xf[:], xf_d[:, :, sl])
                pf = pos_p.tile([3, NBLK], F32R, tag="p3f", name=f"p3f_{nb}")
                nc.sync.dma_start(pf[:], p3f_d[:, sl])
                xf0s.append((xf, pf))
            wv_t = wtile(wv_d, [128, 2, K], BF16, "wv")
            wv3_t = wtile(wv3_d, [3, K], BF16, "wv3")
            xb0s = []
            for nb in (0, 1):
                sl = slice(NBLK * nb, NBLK * (nb + 1))
                xb = x0_p.tile([128, 2, NBLK], BF16, tag="xb", name=f"xb_{nb}")
                nc.sync.dma_start(xb[:], xb_d[:, :, sl])
                pb = pos_p.tile([3, NBLK], BF16, tag="p3b", name=f"p3b_{nb}")
                nc.sync.dma_start(pb[:], p3b_d[:, sl])
                xb0s.append((xb, pb))
            pair_tiles = list(zip(xf0s, xb0s))
            # secondary inputs after pair-0 DMAs, in consumer order
            wq_t = wtile(wq_d, [128, 2, K], F32R, "wq")
            wq3_t = wtile(wq3_d, [3, K], F32R, "wq3")
            p3q_t = wts.tile([3, NSLOT * NBLK], F32R, tag="p3q", name="p3q")
            nc.sync.dma_start(p3q_t[:], p3q_d[:])
            xq_t = []
            for s in range(NSLOT):
                t = xq_p.tile([128, 2, NBLK], F32R, tag=f"xq{s}",
                              name=f"xq{s}")
                nc.sync.dma_start(t[:], xq_d[:, :, NBLK * s:NBLK * (s + 1)])
                xq_t.append(t)
                if s == 0:
                    # mask sets + ones right behind slot-0 critical inputs
                    msk_t = msk_p.tile([128, 2, 8, NBLK], BF16, tag="mk",
                                       name="msk")
                    nc.sync.dma_start(msk_t[:], msk_d[:])
                    ones_b = wtile(ob_d, [128, 128], F32R, "ones_b")

            emit_pair_compute(0, pair_tiles)
            emit_q(0, xq_t[0])

            fcw_t = wtile(fcw_d, [128, 2, C], BF16, "fcw")
            fcb_t = wtile(fcb_d, [128, 2], F32, "fcb")

            pending = None
            for s in range(NSLOT):
                if s + 1 < NSLOT:
                    pair_tiles = emit_pair_dmas(s + 1)
                po, pd = emit_slot(s, pending)
                pending = (s, po, pd, xq_t[s])
                if s + 1 < NSLOT:
                    emit_pair_compute(s + 1, pair_tiles)
                    emit_q(s + 1, xq_t[s + 1])
            finalize_slot(*pending)

    nc.compile()
    return nc


def _host_prep(x, q_w, q_b, k_w, k_b, v_w, v_b, fc_w, fc_b):
    """Build the per-core input maps."""
    import ml_dtypes
    f32 = np.float32
    bf16 = ml_dtypes.bfloat16
    n = np.arange(N)
    px = ((n // S) / S).astype(f32)
    py = ((n % S) / S).astype(f32)
    pos3 = np.stack([px, py, np.ones(N, f32)])   # [3, N] (incl bias channel)

    def merge_h(a):  # [256, M] -> [128, 2, M]
        return np.ascontiguousarray(a.reshape(2, 128, -1).transpose(1, 0, 2))

    def w3(w, b):
        # rows 0..1 = pos weight rows, row 2 = bias
        return np.ascontiguousarray(
            np.concatenate([w.astype(f32).T[C:], b.astype(f32)[None, :]], 0))

    # canonical band-mask patterns: T_r[m, n] = (m <= n - 128r)
    mm = np.arange(128)[:, None]
    nn = np.arange(NBLK)[None, :]
    T = [(mm <= nn - 128 * r).astype(f32) for r in range(4)]
    H = np.stack([np.ones((128, NBLK), f32)] * 4 + T)       # [8, 128, 512]
    L = np.stack(T + [np.zeros((128, NBLK), f32)] * 4)      # [8, 128, 512]

    # per-role [2(set=s%2), 8, 128, 512]: role0 slots are L,H,L,H;
    # role1 slots are H,L,H,L
    masks = {0: np.stack([L, H]), 1: np.stack([H, L])}
    # reorder to [128, 2, 8, NBLK] partition-major for a single DMA
    masks = {r: np.ascontiguousarray(
        m.transpose(2, 0, 1, 3)).astype(bf16) for r, m in masks.items()}

    shared = {
        "wq": merge_h(q_w.astype(f32).T[:C]),
        "wk": merge_h(k_w.astype(f32).T[:C]),
        "wv": merge_h(v_w.astype(f32).T[:C]).astype(bf16),
        "wq3": w3(q_w, q_b), "wk3": w3(k_w, k_b),
        "wv3": w3(v_w, v_b).astype(bf16),
        "p3f": pos3, "p3b": pos3.astype(bf16),
        "fcw": merge_h(fc_w.astype(f32).T).astype(bf16),
        "fcb": np.ascontiguousarray(fc_b.astype(f32).reshape(2, 128).T),
        "ones_b": np.ones((128, 128), f32),
    }

    in_maps = []
    for c in range(8):
        b, r = c // 2, c % 2
        xb = x[b].reshape(C, N).astype(f32)
        xq_cols = np.concatenate(
            [np.arange(NBLK * j, NBLK * (j + 1)) for j in BLOCKS[r]])
        in_maps.append(dict(
            shared,
            xf=merge_h(xb),
            xb=merge_h(xb).astype(bf16),
            xq=merge_h(xb[:, xq_cols]),
            p3q=np.ascontiguousarray(pos3[:, xq_cols]),
            masks=masks[r],
        ))
    return in_maps


def _gather(results):
    out = np.empty((B, C, N), np.float32)
    for c in range(8):
        b, r = c // 2, c % 2
        oc = results[c]["out"]
        for s, j in enumerate(BLOCKS[r]):
            out[b][:, NBLK * j:NBLK * (j + 1)] = oc[:, NBLK * s:NBLK * (s + 1)]
    return out.reshape(B, C, S, S)


def run(trace=False, **inputs):
    from concourse import bass_utils
    global _PROGRAM
    if _PROGRAM is None:
        _PROGRAM = _build_program()
    in_maps = _host_prep(**inputs)
    res = bass_utils.run_bass_kernel_spmd(
        _PROGRAM, in_maps, list(range(8)), trace=trace)
    return _gather(res.results), res


def kernel(**inputs):
    out, _ = run(trace=False, **inputs)
    return out


# revision 17
# speedup vs baseline: 1.1015x; 1.1015x over previous
"""Trainium2 Bass kernel for the AttentionBlock problem.

Reference semantics (shapes hardcoded):
    x [4, 256, 64, 64]; 1x1-conv weights q_w/k_w/v_w [256, 258] (+biases),
    fc_w [256, 256], fc_b [256].
    x0 = concat(x, pos) -> [B, 258, 4096]
    q/k/v = relu(W @ x0 + b)                    [B, 256, 4096]
    attn  = softmax_causal(q^T k)               [B, 4096, 4096]
    out   = x + relu(fc_w @ (attn @ v^T)^T + fc_b)

Distribution: 8 cores = 4 batches x 2 query-block roles. Each core
computes full k / v^T for its batch, q only for its 4 owned 512-wide
query blocks, and causal attention for those blocks. Causal work is
balanced by giving role 0 global blocks [0,3,4,7] and role 1 blocks
[1,2,5,6]; both roles run the identical SPMD program with per-slot
key-tile counts [8,16,24,32] (slightly padded), with per-core mask
data zeroing padded/non-causal entries.

Per-slot band masks take only two canonical [8,128,512] patterns:
H = [1,1,1,1,T0..T3] (slot is exact) and L = [T0..T3,0,0,0,0] (slot
is padded by 4 tiles); for both roles the pattern alternates with
slot parity, so per-core mask data is just [2, 8, 128, 512] indexed
by s%2 (H/L order role-dependent), resident in SBUF.

Softmax is computed without max-subtraction (scores are ~26+-5, far
from fp32 overflow): p = exp(s) * mask, normalized by a replicated
ones-matmul denominator (the [128,128] ones lhsT yields the column
sums broadcast across all partitions). Per-key-tile exps are summed
on VectorE (bf16 quad sums + f32 running total), so only ONE ones
matmul per slot runs on the PE.

Precision split: the score path (q/k projections, q^T k) runs in
float32r; everything whose error enters linearly (exp(p) weights, v,
fc, masks) runs in bf16. Channel-half pairs of x/weights are packed
host-side into [128, 2, *] arrays so each k/v/q input tile is one
DMA. The pos rows (px/py/bias) live in resident [3, N] tiles; the
pos+bias contribution of each projection is a 3-row matmul.

Emission interleaves phases (pair0, q0, slot0, pair1, q1, slot1, ...)
so the PE always has runway while later pair DMAs land. v relus run
on VectorE to keep ScalarE off the phase-A critical path.
"""

import numpy as np

B = 4
C = 256
S = 64
N = S * S            # 4096
K = 256              # q/k/v channels
NBLK = 512           # query block width
NSLOT = 4            # owned query blocks per core
M_S = (8, 16, 24, 32)  # key-tile count per slot (128-wide key tiles)
BLOCKS = ((0, 3, 4, 7), (1, 2, 5, 6))  # role -> global block ids

_PROGRAM = None


def _build_program():
    import concourse.bacc as bacc
    import concourse.mybir as mybir
    import concourse.tile as tile

    F32 = mybir.dt.float32
    F32R = mybir.dt.float32r
    BF16 = mybir.dt.bfloat16
    Act = mybir.ActivationFunctionType

    nc = bacc.Bacc("TRN2", target_bir_lowering=False, debug=False)

    # channel-half-merged inputs: [...][h][...] picks channel half h
    xb_d = nc.dram_tensor("xb", [128, 2, N], BF16, kind="ExternalInput")
    xq_d = nc.dram_tensor("xq", [128, 2, NSLOT * NBLK], BF16,
                          kind="ExternalInput")
    p3b_d = nc.dram_tensor("p3b", [3, N], BF16, kind="ExternalInput")
    p3q_d = nc.dram_tensor("p3q", [3, NSLOT * NBLK], BF16,
                           kind="ExternalInput")
    wq_d = nc.dram_tensor("wq", [128, 2, K], BF16, kind="ExternalInput")
    wk_d = nc.dram_tensor("wk", [128, 2, K], BF16, kind="ExternalInput")
    wv_d = nc.dram_tensor("wv", [128, 2, K], BF16, kind="ExternalInput")
    wq3_d = nc.dram_tensor("wq3", [3, K], BF16, kind="ExternalInput")
    wk3_d = nc.dram_tensor("wk3", [3, K], BF16, kind="ExternalInput")
    wv3_d = nc.dram_tensor("wv3", [3, K], BF16, kind="ExternalInput")
    fcw_d = nc.dram_tensor("fcw", [128, 2, C], BF16, kind="ExternalInput")
    fcb_d = nc.dram_tensor("fcb", [128, 2], F32, kind="ExternalInput")
    msk_d = nc.dram_tensor("masks", [128, 2, 8, NBLK], BF16,
                           kind="ExternalInput")
    ob_d = nc.dram_tensor("ones_b", [128, 128], F32R, kind="ExternalInput")
    out_d = nc.dram_tensor("out", [C, NSLOT * NBLK], F32, kind="ExternalOutput")

    with tile.TileContext(nc) as tc:
        with (
            tc.tile_pool(name="wts", bufs=1) as wts,
            tc.tile_pool(name="pos_p", bufs=8) as pos_p,
            tc.tile_pool(name="x0_p", bufs=8) as x0_p,
            tc.tile_pool(name="xq_p", bufs=1) as xq_p,
            tc.tile_pool(name="kqv_p", bufs=1) as kqv_p,
            tc.tile_pool(name="msk_p", bufs=1) as msk_p,
            tc.tile_pool(name="ex_p", bufs=9) as ex_p,
            tc.tile_pool(name="ds_p", bufs=3) as ds_p,
            tc.tile_pool(name="tot_p", bufs=2) as tot_p,
            tc.tile_pool(name="o_p", bufs=4) as o_p,
            tc.tile_pool(name="rb_p", bufs=2) as rb_p,
            tc.tile_pool(name="tr_p", bufs=2) as tr_p,
            tc.tile_pool(name="ps_sc", bufs=5, space="PSUM") as ps_sc,
            tc.tile_pool(name="ps_out", bufs=1, space="PSUM") as ps_out,
            tc.tile_pool(name="ps_mx", bufs=1, space="PSUM") as ps_mx,
        ):
            def wtile(dram, shape, dt, tag):
                t = wts.tile(shape, dt, tag=tag, name=tag)
                nc.sync.dma_start(t[:], dram[:])
                return t

            k_sb = [[None] * 8 for _ in range(2)]
            vT_sb = [None] * 32
            q_sb = [[None] * NSLOT for _ in range(2)]

            def emit_block_dmas(nb):
                sl = slice(NBLK * nb, NBLK * (nb + 1))
                xb = x0_p.tile([128, 2, NBLK], BF16, tag="xb",
                               name=f"xb_{nb}")
                nc.sync.dma_start(xb[:], xb_d[:, :, sl])
                pb = pos_p.tile([3, NBLK], BF16, tag="p3b",
                                name=f"p3b_{nb}")
                nc.sync.dma_start(pb[:], p3b_d[:, sl])
                return (xb, pb)

            def emit_pair_compute(nbp, tiles):
                for li, nb in enumerate((2 * nbp, 2 * nbp + 1)):
                    xb, pb = tiles[li]
                    for kt in range(2):
                        kts = slice(128 * kt, 128 * (kt + 1))
                        pk = ps_sc.tile([128, NBLK], F32, tag="sc",
                                        name=f"pk{kt}_{nb}")
                        nc.tensor.matmul(pk[:], wk_t[:, 0, kts], xb[:, 0, :],
                                         start=True, stop=False)
                        nc.tensor.matmul(pk[:], wk_t[:, 1, kts], xb[:, 1, :],
                                         start=False, stop=False)
                        nc.tensor.matmul(pk[:], wk3_t[:, kts], pb[:],
                                         start=False, stop=True)
                        kt_sb = kqv_p.tile([128, NBLK], F32R,
                                           tag=f"k{kt}_{nb}",
                                           name=f"k{kt}_{nb}")
                        nc.scalar.activation(kt_sb[:], pk[:], Act.Relu)
                        k_sb[kt][nb] = kt_sb
                for li, nb in enumerate((2 * nbp, 2 * nbp + 1)):
                    xb, pb = tiles[li]
                    for sub in range(4):
                        i = 4 * nb + sub
                        ss = slice(128 * sub, 128 * (sub + 1))
                        pv = ps_sc.tile([128, NBLK], F32, tag="sc",
                                        name=f"pv{i}")
                        nc.tensor.matmul(pv[:, :K], xb[:, 0, ss], wv_t[:, 0, :],
                                         start=True, stop=False)
                        nc.tensor.matmul(pv[:, :K], xb[:, 1, ss], wv_t[:, 1, :],
                                         start=False, stop=False)
                        nc.tensor.matmul(pv[:, :K], pb[:, ss], wv3_t[:],
                                         start=False, stop=True)
                        vt_sb = kqv_p.tile([128, K], BF16, tag=f"v{i}",
                                           name=f"v{i}")
                        nc.vector.tensor_scalar_max(vt_sb[:], pv[:, :K], 0.0)
                        vT_sb[i] = vt_sb

            def emit_q(s, xqs):
                sl = slice(NBLK * s, NBLK * (s + 1))
                for kt in range(2):
                    kts = slice(128 * kt, 128 * (kt + 1))
                    pq = ps_sc.tile([128, NBLK], F32, tag="sc",
                                    name=f"pq{kt}_{s}")
                    nc.tensor.matmul(pq[:], wq_t[:, 0, kts], xqs[:, 0, :],
                                     start=True, stop=False)
                    nc.tensor.matmul(pq[:], wq_t[:, 1, kts], xqs[:, 1, :],
                                     start=False, stop=False)
                    nc.tensor.matmul(pq[:], wq3_t[:, kts], p3q_t[:, sl],
                                     start=False, stop=True)
                    qt = kqv_p.tile([128, NBLK], F32R, tag=f"q{kt}_{s}",
                                    name=f"q{kt}_{s}")
                    nc.scalar.activation(qt[:], pq[:], Act.Relu)
                    q_sb[kt][s] = qt

            def finalize_slot(s, po, pd, xqs):
                """normalize slot s, fc, relu, residual, dma out."""
                rb_sb = rb_p.tile([128, NBLK], F32, tag="rb", name=f"rb{s}")
                nc.vector.reciprocal_approx_fast(rb_sb[:], pd[:])
                o_sb = []
                for vt in range(2):
                    ot = o_p.tile([128, NBLK], BF16, tag="o",
                                  name=f"o{vt}_{s}")
                    nc.vector.tensor_mul(ot[:], po[vt][:], rb_sb[:])
                    o_sb.append(ot)
                for ot in range(2):
                    pfc = ps_mx.tile([128, NBLK], F32, tag="mx",
                                     name=f"pfc{ot}_{s}")
                    for vt in range(2):
                        nc.tensor.matmul(
                            pfc[:], fcw_t[:, vt, 128 * ot:128 * (ot + 1)],
                            o_sb[vt][:], start=(vt == 0), stop=(vt == 1))
                    t_sb = tr_p.tile([128, NBLK], F32, tag=f"t{ot}",
                                     name=f"t{ot}_{s}")
                    nc.scalar.activation(t_sb[:], pfc[:], Act.Relu,
                                         bias=fcb_t[:, ot:ot + 1])
                    r_sb = tr_p.tile([128, NBLK], F32, tag=f"r{ot}",
                                     name=f"r{ot}_{s}")
                    nc.vector.tensor_add(r_sb[:], t_sb[:], xqs[:, ot, :])
                    nc.sync.dma_start(
                        out_d[128 * ot:128 * (ot + 1),
                              NBLK * s:NBLK * (s + 1)], r_sb[:])

            def emit_slot(s, fin):
                M = M_S[s]
                po = [ps_out.tile([128, NBLK], F32, tag=f"o{vt}",
                                  name=f"po{vt}_{s}") for vt in range(2)]
                ex_tiles = [None] * M
                tot = [None]

                def emit_scores(i):
                    # scores^T tile [128 keys, 512 queries]
                    psc = ps_sc.tile([128, NBLK], F32, tag="sc",
                                     name=f"psc{s}_{i}")
                    for kt in range(2):
                        nc.tensor.matmul(
                            psc[:],
                            k_sb[kt][i // 4][:, 128 * (i % 4):128 * (i % 4 + 1)],
                            q_sb[kt][s][:], start=(kt == 0), stop=(kt == 1))
                    ex = ex_p.tile([128, NBLK], BF16, tag="ex",
                                   name=f"ex{s}_{i}")
                    nc.scalar.activation(ex[:], psc[:], Act.Exp)
                    if i >= M - 8:
                        nc.vector.tensor_mul(
                            ex[:], ex[:], msk_t[:, s % 2, i - (M - 8), :])
                    ex_tiles[i] = ex

                def consume_quad(j):
                    for jj in range(j, j + 4):
                        e = ex_tiles[jj]
                        for vt in range(2):
                            nc.tensor.matmul(
                                po[vt][:],
                                vT_sb[jj][:, 128 * vt:128 * (vt + 1)],
                                e[:], start=(jj == 0), stop=(jj == M - 1))
                    # bf16 quad sum, then f32 running total on VectorE
                    da = ds_p.tile([128, NBLK], BF16, tag="ds",
                                   name=f"da{s}_{j}")
                    nc.vector.tensor_add(da[:], ex_tiles[j][:],
                                         ex_tiles[j + 1][:])
                    db = ds_p.tile([128, NBLK], BF16, tag="ds",
                                   name=f"db{s}_{j}")
                    nc.vector.tensor_add(db[:], ex_tiles[j + 2][:],
                                         ex_tiles[j + 3][:])
                    t = tot_p.tile([128, NBLK], F32R, tag="tot",
                                   name=f"tot{s}_{j}")
                    if tot[0] is None:
                        nc.vector.tensor_add(t[:], da[:], db[:])
                    else:
                        dsum = ds_p.tile([128, NBLK], BF16, tag="ds",
                                         name=f"ds{s}_{j}")
                        nc.vector.tensor_add(dsum[:], da[:], db[:])
                        nc.vector.tensor_add(t[:], tot[0][:], dsum[:])
                    tot[0] = t
                    for jj in range(j, j + 4):
                        ex_tiles[jj] = None

                # 4-tile score batches between bf16 consume batches; the
                # previous slot's finalize is deferred to after the second
                # score batch so VectorE has runway for recip+muls
                for ib in range(0, M, 4):
                    for i in range(ib, ib + 4):
                        emit_scores(i)
                    if ib == 4 and fin is not None:
                        finalize_slot(*fin)
                    if ib >= 4:
                        consume_quad(ib - 4)
                consume_quad(M - 4)
                # single denominator matmul per slot
                pd = ps_mx.tile([128, NBLK], F32, tag="mx", name=f"pd{s}")
                nc.tensor.matmul(pd[:], ones_b[:], tot[0][:],
                                 start=True, stop=True)
                return po, pd

            # ---------------- emission schedule ----------------
            # front-loaded DMAs in first-consumer order; everything fits
            # in SBUF so transfers complete early and stop competing with
            # the PE for SBUF bandwidth
            wk_t = wtile(wk_d, [128, 2, K], BF16, "wk")
            wk3_t = wtile(wk3_d, [3, K], BF16, "wk3")
            blk_tiles = [emit_block_dmas(0), emit_block_dmas(1)]
            wv_t = wtile(wv_d, [128, 2, K], BF16, "wv")
            wv3_t = wtile(wv3_d, [3, K], BF16, "wv3")
            wq_t = wtile(wq_d, [128, 2, K], BF16, "wq")
            wq3_t = wtile(wq3_d, [3, K], BF16, "wq3")
            p3q_t = wts.tile([3, NSLOT * NBLK], BF16, tag="p3q", name="p3q")
            nc.sync.dma_start(p3q_t[:], p3q_d[:])
            xq_t = []
            for s in range(NSLOT):
                t = xq_p.tile([128, 2, NBLK], BF16, tag=f"xq{s}",
                              name=f"xq{s}")
                nc.sync.dma_start(t[:], xq_d[:, :, NBLK * s:NBLK * (s + 1)])
                xq_t.append(t)
            msk_t = msk_p.tile([128, 2, 8, NBLK], BF16, tag="mk", name="msk")
            nc.sync.dma_start(msk_t[:], msk_d[:])
            ones_b = wtile(ob_d, [128, 128], F32R, "ones_b")
            for nb in range(2, 8):
                blk_tiles.append(emit_block_dmas(nb))
            fcw_t = wtile(fcw_d, [128, 2, C], BF16, "fcw")
            fcb_t = wtile(fcb_d, [128, 2], F32, "fcb")

            emit_pair_compute(0, blk_tiles[0:2])
            emit_q(0, xq_t[0])

            pending = None
            for s in range(NSLOT):
                po, pd = emit_slot(s, pending)
                pending = (s, po, pd, xq_t[s])
                if s + 1 < NSLOT:
                    emit_pair_compute(s + 1, blk_tiles[2 * s + 2:2 * s + 4])
                    emit_q(s + 1, xq_t[s + 1])
            finalize_slot(*pending)

    nc.compile()
    return nc


def _host_prep(x, q_w, q_b, k_w, k_b, v_w, v_b, fc_w, fc_b):
    """Build the per-core input maps."""
    import ml_dtypes
    f32 = np.float32
    bf16 = ml_dtypes.bfloat16
    n = np.arange(N)
    px = ((n // S) / S).astype(f32)
    py = ((n % S) / S).astype(f32)
    pos3 = np.stack([px, py, np.ones(N, f32)])   # [3, N] (incl bias channel)

    def merge_h(a):  # [256, M] -> [128, 2, M]
        return np.ascontiguousarray(a.reshape(2, 128, -1).transpose(1, 0, 2))

    def w3(w, b):
        # rows 0..1 = pos weight rows, row 2 = bias
        return np.ascontiguousarray(
            np.concatenate([w.astype(f32).T[C:], b.astype(f32)[None, :]], 0))

    # canonical band-mask patterns: T_r[m, n] = (m <= n - 128r)
    mm = np.arange(128)[:, None]
    nn = np.arange(NBLK)[None, :]
    T = [(mm <= nn - 128 * r).astype(f32) for r in range(4)]
    H = np.stack([np.ones((128, NBLK), f32)] * 4 + T)       # [8, 128, 512]
    L = np.stack(T + [np.zeros((128, NBLK), f32)] * 4)      # [8, 128, 512]

    # per-role [2(set=s%2), 8, 128, 512]: role0 slots are L,H,L,H;
    # role1 slots are H,L,H,L
    masks = {0: np.stack([L, H]), 1: np.stack([H, L])}
    # reorder to [128, 2, 8, NBLK] partition-major for a single DMA
    masks = {r: np.ascontiguousarray(
        m.transpose(2, 0, 1, 3)).astype(bf16) for r, m in masks.items()}

    shared = {
        "wq": merge_h(q_w.astype(f32).T[:C]).astype(bf16),
        "wk": merge_h(k_w.astype(f32).T[:C]).astype(bf16),
        "wv": merge_h(v_w.astype(f32).T[:C]).astype(bf16),
        "wq3": w3(q_w, q_b).astype(bf16), "wk3": w3(k_w, k_b).astype(bf16),
        "wv3": w3(v_w, v_b).astype(bf16),
        "p3b": pos3.astype(bf16),
        "fcw": merge_h(fc_w.astype(f32).T).astype(bf16),
        "fcb": np.ascontiguousarray(fc_b.astype(f32).reshape(2, 128).T),
        "ones_b": np.ones((128, 128), f32),
    }

    in_maps = []
    for c in range(8):
        b, r = c // 2, c % 2
        xb = x[b].reshape(C, N).astype(f32)
        xq_cols = np.concatenate(
            [np.arange(NBLK * j, NBLK * (j + 1)) for j in BLOCKS[r]])
        in_maps.append(dict(
            shared,
            xb=merge_h(xb).astype(bf16),
            xq=merge_h(xb[:, xq_cols]).astype(bf16),
            p3q=np.ascontiguousarray(pos3[:, xq_cols]).astype(bf16),
            masks=masks[r],
        ))
    return in_maps


def _gather(results):
    out = np.empty((B, C, N), np.float32)
    for c in range(8):
        b, r = c // 2, c % 2
        oc = results[c]["out"]
        for s, j in enumerate(BLOCKS[r]):
            out[b][:, NBLK * j:NBLK * (j + 1)] = oc[:, NBLK * s:NBLK * (s + 1)]
    return out.reshape(B, C, S, S)


def run(trace=False, **inputs):
    from concourse import bass_utils
    global _PROGRAM
    if _PROGRAM is None:
        _PROGRAM = _build_program()
    in_maps = _host_prep(**inputs)
    res = bass_utils.run_bass_kernel_spmd(
        _PROGRAM, in_maps, list(range(8)), trace=trace)
    return _gather(res.results), res


def kernel(**inputs):
    out, _ = run(trace=False, **inputs)
    return out
